# revision 1
# baseline (speedup 1.0000x reference)
"""BrainGNNEncoder (3-layer GCN + BN + ReLU + mean/sum graph pooling) on 8 TRN2 NeuronCores.

Pipeline (per core, SPMD — identical program, per-core data):
  * nodes sharded contiguously; edges sharded by destination node.
  * GCN normalization + BatchNorm folded on the host into per-edge `norm`
    values and folded weights W'' / biases b''.
  * per layer:
      dense:  psum2[node, ch] = hT[ch_in, node].T @ W''            (PE)
      writeback bf16 shard -> AllGather -> node table [100352, 128] bf16
      aggregate: dma_gather (int16 idx, 4 SWDGE queues, 4 source windows
        of 32k rows) fetches source rows; per 128-edge block a matmul
        msgs[edge, ch].T @ S[edge, 128] accumulates into psum1[ch, dst].
        S is built on-device by DVE from per-slot (dstcol, norm) via an
        iota-compare (one-hot * norm).
      ScalarE: hT = Relu(psum1 + b'')   (fused BN shift + bias + ReLU);
        last layer also emits per-supergroup column sums via accum_out
        (graph pooling partials).
  * host combines pooling partials (boundary supergroups re-summed from the
    h3 output) into [G, 2H] mean||sum.
"""

import hashlib
import numpy as np
import ml_dtypes

import concourse.bass as bass
import concourse.bacc as bacc
import concourse.tile as tile
import concourse.mybir as mybir
from concourse import library_config
from concourse.bass_utils import run_bass_kernel_spmd

BF16 = ml_dtypes.bfloat16
P = 128            # edge slots per block (matmul contraction dim)
EL = 128           # table row width (bf16) = 256B; cols [H:] are zero pad
NCORES = 8
EPS = 1e-5
SRCWIN = 32768     # rows per source window (int16 index range)
MAXBLK_GATHER = 8  # blocks per dma_gather (num_idxs <= 1024)
NQ = 4             # SWDGE queues


class Plan:
    pass


# ----------------------------------------------------------------------------
# Host-side plan
# ----------------------------------------------------------------------------

def make_plan(x, edge_index, edge_weight, batch, Ws, bs, gammas, betas, rms, rvs,
              srcwin=None):
    pl = Plan()
    N, IN = x.shape
    H = Ws[0].shape[1]
    L = len(Ws)
    G = int(batch.max()) + 1 if batch.size else 1
    assert N % NCORES == 0
    NLOC = N // NCORES
    SGN = (NLOC + P - 1) // P
    NLOC_PAD = SGN * P
    TROWS = NLOC_PAD * NCORES
    if srcwin is None:
        # equal-size source windows (int16 limit 32767 rows per window)
        nsw0 = (TROWS + SRCWIN - 1) // SRCWIN
        srcwin = (TROWS + nsw0 - 1) // nsw0
    NSW = (TROWS + srcwin - 1) // srcwin
    pl.N, pl.IN, pl.H, pl.L, pl.G = N, IN, H, L, G
    pl.NLOC, pl.SGN, pl.NLOC_PAD, pl.TROWS, pl.NSW = NLOC, SGN, NLOC_PAD, TROWS, NSW
    pl.srcwin = srcwin

    # ---- BN folding ----
    Wpp, bpp = [], []
    for l in range(L):
        alpha = (gammas[l] / np.sqrt(rvs[l] + EPS)).astype(np.float32)
        Wpp.append((Ws[l] * alpha[None, :]).astype(np.float32))
        bpp.append(((bs[l] - rms[l]) * alpha + betas[l]).astype(np.float32))
    pl.Wpp = Wpp
    pl.bias_host = np.stack(bpp, axis=1).astype(np.float32)       # [H, L]

    # ---- edges + self loops, symmetric norm ----
    src = np.concatenate([edge_index[0], np.arange(N, dtype=np.int64)])
    dst = np.concatenate([edge_index[1], np.arange(N, dtype=np.int64)])
    w = np.concatenate([np.abs(edge_weight), np.ones(N, np.float32)]).astype(np.float32)
    deg = np.zeros(N, np.float32)
    np.add.at(deg, dst, w)
    dinv = np.where(deg > 0,
                    1.0 / np.sqrt(np.where(deg > 0, deg, 1.0)), 0.0).astype(np.float32)
    norm = (dinv[src] * w * dinv[dst]).astype(np.float32)

    core_of = dst // NLOC
    jloc = dst - core_of * NLOC          # local dst id
    sg_of = jloc // P
    # table row (p-major within each core block)
    q2 = src // NLOC
    r2 = src % NLOC
    trow = q2 * NLOC_PAD + SGN * (r2 % P) + (r2 // P)
    sig_of = trow // srcwin

    # cell = (core, sg, sigma); sort edges by (core, sg, sigma)
    NCELL = SGN * NSW
    cellid = sg_of * NSW + sig_of
    counts = np.zeros((NCORES, NCELL), np.int64)
    for q in range(NCORES):
        m = core_of == q
        counts[q] = np.bincount(cellid[m], minlength=NCELL)
    rblk = ((counts.max(axis=0) + P - 1) // P).astype(np.int64)     # [NCELL]
    blk_base = np.concatenate([[0], np.cumsum(rblk)])[:-1]
    TOTBLK = int(rblk.sum())
    pl.rblk, pl.blk_base, pl.TOTBLK = rblk, blk_base, TOTBLK

    # gather instructions: per cell, chunks of <= MAXBLK_GATHER blocks
    instrs = []   # (sigma, blk0, nblk, idxcol0)
    idxcol = 0
    for sg in range(SGN):
        for sig in range(NSW):
            ci = sg * NSW + sig
            nb = int(rblk[ci])
            b0 = int(blk_base[ci])
            off = 0
            while off < nb:
                step = min(MAXBLK_GATHER, nb - off)
                instrs.append((sig, b0 + off, step, idxcol))
                idxcol += step * 8
                off += step
    pl.instrs = instrs
    pl.IDXCOLS = idxcol

    # blocks of each supergroup (contiguous range)
    pl.sg_blk = [(int(blk_base[sg * NSW]),
                  int(blk_base[(sg + 1) * NSW - 1] + rblk[(sg + 1) * NSW - 1]))
                 for sg in range(SGN)]

    # ---- per-core packed data ----
    pl.idx_data, pl.colv_data, pl.normv_data, pl.xT_data = [], [], [], []
    order = np.lexsort((jloc, cellid, core_of))   # sort by core, cell, dst
    src_s = trow[order]
    col_s = (jloc % P)[order]
    norm_s = norm[order]
    cell_s = cellid[order]
    core_s = core_of[order]
    for q in range(NCORES):
        m = core_s == q
        cq, rq, colq, nq = cell_s[m], src_s[m], col_s[m], norm_s[m]
        off = np.concatenate([[0], np.cumsum(counts[q])])[:-1]
        pos = np.arange(len(cq)) - off[cq]
        blk_e = blk_base[cq] + pos // P
        p_e = pos % P

        # pads fetch row 0 of the window (cheap, and guarantees finite data
        # in every consumed slot; S rows are zero there)
        idx16 = np.zeros((TOTBLK, P), np.int16)
        idx16[blk_e, p_e] = (rq - (rq // srcwin) * srcwin).astype(np.int16)
        colv = np.zeros((TOTBLK, P), np.float32)
        colv[blk_e, p_e] = colq
        normv = np.zeros((TOTBLK, P), np.float32)
        normv[blk_e, p_e] = nq

        # wrapped idx stream per instruction
        idxw = np.zeros((P, idxcol), np.int16)
        for (sig, b0, nblk, c0) in instrs:
            flat = idx16[b0:b0 + nblk].reshape(-1)       # pos i -> (blk i//128, p i%128)
            idxw[:, c0:c0 + nblk * 8] = np.tile(
                flat.reshape(nblk * 8, 16).T, (NCORES, 1))
        pl.idx_data.append(idxw)
        pl.colv_data.append(np.ascontiguousarray(colv.T).astype(BF16))   # [P, TOTBLK]
        pl.normv_data.append(np.ascontiguousarray(normv.T).astype(BF16))

        xs = x[q * NLOC:(q + 1) * NLOC]
        xT = np.zeros((IN, NLOC_PAD), np.float32)
        xT[:, :NLOC] = xs.T
        pl.xT_data.append(xT.astype(BF16))

    # ---- pooling bookkeeping ----
    gcnt = np.bincount(batch, minlength=G).astype(np.int64)
    gstart = np.concatenate([[0], np.cumsum(gcnt)])
    pl.gcnt = gcnt
    pieces = []
    for q in range(NCORES):
        lo = q * NLOC
        per_sg = []
        for sg in range(SGN):
            s0, s1 = sg * P, min(sg * P + P, NLOC)
            segs = []
            c = s0
            g0 = int(np.searchsorted(gstart, lo + c, side="right") - 1)
            while c < s1:
                g_end = int(gstart[g0 + 1]) - lo
                e = min(s1, g_end)
                segs.append((c - s0, e - s0, g0))
                c = e
                if c >= g_end:
                    g0 += 1
            per_sg.append(segs)
        pieces.append(per_sg)
    pl.pieces = pieces
    return pl


# ----------------------------------------------------------------------------
# Program builder
# ----------------------------------------------------------------------------

def build_program(pl):
    dt = mybir.dt
    f32, bf16, i16 = dt.float32, dt.bfloat16, dt.int16
    IN, H, SGN, NLOC_PAD, TOTBLK, L = pl.IN, pl.H, pl.SGN, pl.NLOC_PAD, pl.TOTBLK, pl.L
    TROWS = pl.TROWS

    nc = bacc.Bacc("TRN2", target_bir_lowering=False, debug=False,
                   num_devices=NCORES, num_swdge_queues=NQ)

    xT_d = nc.dram_tensor("xT", [IN, NLOC_PAD], bf16, kind="ExternalInput")
    idx_d = nc.dram_tensor("idx", [P, pl.IDXCOLS], i16, kind="ExternalInput")
    colv_d = nc.dram_tensor("colv", [P, TOTBLK], bf16, kind="ExternalInput")
    normv_d = nc.dram_tensor("normv", [P, TOTBLK], bf16, kind="ExternalInput")
    iota_d = nc.dram_tensor("iota", [P, P], bf16, kind="ExternalInput")
    W_d = [nc.dram_tensor(f"W{l}", [IN if l == 0 else H, H], bf16,
                          kind="ExternalInput") for l in range(L)]
    bias_d = nc.dram_tensor("bias", [H, L], f32, kind="ExternalInput")
    pool_d = nc.dram_tensor("pool", [H, SGN], f32, kind="ExternalOutput")
    h3_d = nc.dram_tensor("h3", [H, NLOC_PAD], bf16, kind="ExternalOutput")

    rg = [list(range(NCORES))]

    with tile.TileContext(nc) as tc:
        with (
            tc.tile_pool(name="const", bufs=1) as constp,
            tc.tile_pool(name="xtp", bufs=1) as xtp,
            tc.tile_pool(name="dram", bufs=1, space="DRAM") as dramp,
            tc.tile_pool(name="msgs", bufs=6) as msgp,
            tc.tile_pool(name="sbld", bufs=3) as sbldp,
            tc.tile_pool(name="aggp", bufs=4) as aggp,
            tc.tile_pool(name="ps1p", bufs=4, space="PSUM") as ps1p,
            tc.tile_pool(name="ps2p", bufs=4, space="PSUM") as ps2p,
        ):
            nc.gpsimd.load_library(library_config.mlp)
            # ---------------- constants ----------------
            idx_sb = constp.tile([P, pl.IDXCOLS], i16, name="idx_sb", tag="idx_sb")
            nc.sync.dma_start(out=idx_sb[:], in_=idx_d[:, :])
            colv_sb = constp.tile([P, TOTBLK], bf16, name="colv_sb", tag="colv_sb")
            nc.sync.dma_start(out=colv_sb[:], in_=colv_d[:, :])
            normv_sb = constp.tile([P, TOTBLK], bf16, name="normv_sb", tag="normv_sb")
            nc.sync.dma_start(out=normv_sb[:], in_=normv_d[:, :])
            iota_sb = constp.tile([P, P], bf16, name="iota_sb", tag="iota_sb")
            nc.sync.dma_start(out=iota_sb[:], in_=iota_d[:, :])
            W_sb = []
            for l in range(L):
                wt = constp.tile([IN if l == 0 else H, H], bf16,
                                 name=f"W{l}_sb", tag=f"W{l}_sb")
                nc.sync.dma_start(out=wt[:], in_=W_d[l][:])
                W_sb.append(wt)
            bias_sb = constp.tile([H, L], f32, name="bias_sb", tag="bias_sb")
            nc.sync.dma_start(out=bias_sb[:], in_=bias_d[:])
            xT_sb = xtp.tile([IN, NLOC_PAD], bf16, name="xT_sb", tag="xT_sb")
            nc.sync.dma_start(out=xT_sb[:], in_=xT_d[:, :])

            wb = constp.tile([P, SGN * H], bf16, name="wb", tag="wb")
            h3T = constp.tile([H, NLOC_PAD], bf16, name="h3T", tag="h3T")
            pool_sb = constp.tile([H, SGN], f32, name="pool_sb", tag="pool_sb")
            nc.vector.memset(h3T[:], 0.0)
            nc.vector.memset(pool_sb[:], 0.0)

            bounce = [dramp.tile([NLOC_PAD, EL], bf16, name=f"bounce{l}",
                                 tag=f"bounce{l}") for l in range(L)]
            tables = [dramp.tile([TROWS, EL], bf16, addr_space="Shared",
                                 name=f"T{l}", tag=f"T{l}") for l in range(L)]

            def writeback_and_allgather(l):
                dview = bounce[l].rearrange("(p s) h -> p s h", p=P)[:, :, :H]
                nc.sync.dma_start(
                    out=dview, in_=wb[:].rearrange("p (s h) -> p s h", h=H))
                nc.gpsimd.collective_compute(
                    "AllGather", mybir.AluOpType.bypass,
                    replica_groups=rg,
                    ins=[bounce[l][:, :].opt()],
                    outs=[tables[l][:, :].opt()],
                )

            # wb holds [P, SGN, H] -> bounce rows 98p+sg hold h of node 128sg+p,
            # but bounce rows are EL wide; wb writes only H cols per node.
            # Map: bounce viewed [P, SGN, EL]; write [:, :, :H], zero the rest
            # once via memset of the DRAM tile? DMA writes only H cols; pad
            # cols stay whatever DRAM had. Gathered pad cols feed lhsT slice
            # [:, :H] only — so pad cols are never consumed. (lhsT reads :H.)

            # ---------------- phase D0: dense layer 0 ----------------
            for t in range(SGN):
                ps2 = ps2p.tile([P, H], f32, tag="ps2", name=f"ps2_d0_{t}")
                nc.tensor.matmul(out=ps2[:], lhsT=xT_sb[:, t * P:(t + 1) * P],
                                 rhs=W_sb[0][:], start=True, stop=True)
                nc.vector.tensor_copy(out=wb[:, t * H:(t + 1) * H], in_=ps2[:])
            writeback_and_allgather(0)

            # ---------------- layers ----------------
            import os as _os
            MAXL = int(_os.environ.get("KERNEL_MAXL", str(L)))
            NO_GATHER = _os.environ.get("KERNEL_NO_GATHER") == "1"
            NO_SBUILD = _os.environ.get("KERNEL_NO_SBUILD") == "1"
            NO_AGGMM = _os.environ.get("KERNEL_NO_AGGMM") == "1"
            for l in range(min(L, MAXL)):
                Tl = tables[l]
                gi = 0           # next gather instruction to issue
                msgs_of = {}     # blk -> (tile, chunk)
                for sg in range(SGN):
                    b_lo, b_hi = pl.sg_blk[sg]
                    # issue gathers covering [b_lo, b_hi)
                    while gi < len(pl.instrs) and pl.instrs[gi][1] < b_hi:
                        sig, b0, nblk, c0 = pl.instrs[gi]
                        m = msgp.tile([P, MAXBLK_GATHER, EL], bf16, tag="msgs",
                                      name=f"msgs_{l}_{gi}")
                        if NO_GATHER:
                            nc.vector.memset(m[:1, :1, :1], 0.0)
                        else:
                            nc.gpsimd.dma_gather(
                                out_ap=m[:, :nblk, :],
                                in_ap=Tl[pl.srcwin * sig:, :],
                                idxs_ap=idx_sb[:, c0:c0 + nblk * 8],
                                num_idxs=nblk * P, num_idxs_reg=nblk * P,
                                elem_size=EL, queue_num=gi % NQ)
                        for k in range(nblk):
                            msgs_of[b0 + k] = (m, k)
                        gi += 1
                    nbs = b_hi - b_lo
                    if nbs == 0:
                        continue
                    # build S for this supergroup's blocks on DVE
                    S_sb = sbldp.tile([P, nbs, P], bf16, tag="S_sb",
                                      name=f"S_{l}_{sg}")
                    iota_b = bass.AP(iota_sb[:].tensor, iota_sb[:].offset,
                                     [iota_sb[:].ap[0], [0, nbs],
                                      iota_sb[:].ap[1]])
                    colv_b = colv_sb[:, b_lo:b_hi, None].to_broadcast([P, nbs, P])
                    normv_b = normv_sb[:, b_lo:b_hi, None].to_broadcast([P, nbs, P])
                    if NO_SBUILD:
                        nc.vector.memset(S_sb[:1, :1, :1], 0.0)
                    else:
                        nc.vector.tensor_tensor(out=S_sb[:], in0=iota_b,
                                                in1=colv_b,
                                                op=mybir.AluOpType.is_equal)
                        nc.vector.tensor_tensor(out=S_sb[:], in0=S_sb[:],
                                                in1=normv_b,
                                                op=mybir.AluOpType.mult)
                    ps1 = ps1p.tile([H, P], f32, tag="ps1", name=f"ps1_{l}_{sg}")
                    if NO_AGGMM:
                        nc.vector.memset(ps1[:1, :1], 0.0)
                        for bi in range(nbs):
                            msgs_of.pop(b_lo + bi)
                    else:
                        for bi in range(nbs):
                            m, k = msgs_of.pop(b_lo + bi)
                            nc.tensor.matmul(
                                out=ps1[:, :],
                                lhsT=m[:, k, :H],
                                rhs=S_sb[:, bi, :],
                                start=(bi == 0), stop=(bi == nbs - 1))
                    if l < L - 1:
                        aggT = aggp.tile([H, P], bf16, tag="aggT",
                                         name=f"aggT_{l}_{sg}")
                        nc.scalar.activation(
                            out=aggT[:], in_=ps1[:],
                            func=mybir.ActivationFunctionType.Relu,
                            bias=bias_sb[:, l:l + 1], scale=1.0)
                        ps2 = ps2p.tile([P, H], f32, tag="ps2",
                                        name=f"ps2_{l}_{sg}")
                        nc.tensor.matmul(out=ps2[:], lhsT=aggT[:],
                                         rhs=W_sb[l + 1][:],
                                         start=True, stop=True)
                        nc.vector.tensor_copy(
                            out=wb[:, sg * H:(sg + 1) * H], in_=ps2[:])
                    else:
                        nc.scalar.activation(
                            out=h3T[:, sg * P:(sg + 1) * P], in_=ps1[:],
                            func=mybir.ActivationFunctionType.Relu,
                            bias=bias_sb[:, l:l + 1], scale=1.0,
                            accum_out=pool_sb[:, sg:sg + 1])
                if l < L - 1:
                    writeback_and_allgather(l + 1)

            nc.sync.dma_start(out=pool_d[:, :], in_=pool_sb[:])
            nc.sync.dma_start(out=h3_d[:, :], in_=h3T[:])

    nc.compile()
    return nc


# ----------------------------------------------------------------------------
# kernel entry point
# ----------------------------------------------------------------------------

_CACHE = {}


def _inputs_key(inputs):
    h = hashlib.sha1()
    for k in sorted(inputs.keys()):
        a = np.asarray(inputs[k])
        h.update(k.encode())
        h.update(str(a.shape).encode())
    h.update(np.ascontiguousarray(np.asarray(inputs["edge_index"], np.int64)).tobytes())
    h.update(np.ascontiguousarray(np.asarray(inputs["batch"], np.int64)).tobytes())
    return h.hexdigest()


def _run(pl, nc):
    iota = np.broadcast_to(np.arange(P, dtype=np.float32), (P, P))
    iota = np.ascontiguousarray(iota).astype(BF16)
    in_maps = []
    for q in range(NCORES):
        in_map = {
            "xT": pl.xT_data[q],
            "idx": pl.idx_data[q],
            "colv": pl.colv_data[q],
            "normv": pl.normv_data[q],
            "iota": iota,
            "bias": pl.bias_host,
        }
        for l in range(pl.L):
            in_map[f"W{l}"] = pl.Wpp[l].astype(BF16)
        in_maps.append(in_map)
    return run_bass_kernel_spmd(nc, in_maps, core_ids=list(range(NCORES)))


def _assemble(pl, results):
    G, H, SGN, P_ = pl.G, pl.H, pl.SGN, P
    sums = np.zeros((G, H), np.float64)
    for q in range(NCORES):
        pool = np.asarray(results[q]["pool"], np.float64)   # [H, SGN]
        h3 = np.asarray(results[q]["h3"], np.float32)       # [H, NLOC_PAD]
        for sg in range(SGN):
            segs = pl.pieces[q][sg]
            if len(segs) == 1 and segs[0][0] == 0 and segs[0][1] == P_:
                sums[segs[0][2]] += pool[:, sg]
            else:
                for (c0, c1, g) in segs:
                    sums[g] += h3[:, sg * P_ + c0: sg * P_ + c1].astype(
                        np.float64).sum(axis=1)
    cnt = np.maximum(pl.gcnt, 1).astype(np.float64)
    mean = sums / cnt[:, None]
    return np.concatenate([mean, sums], axis=1).astype(np.float32)


def kernel(**inputs) -> np.ndarray:
    x = np.asarray(inputs["x"], np.float32)
    edge_index = np.asarray(inputs["edge_index"]).astype(np.int64)
    edge_weight = np.asarray(inputs["edge_weight"], np.float32)
    batch = np.asarray(inputs["batch"]).astype(np.int64)
    L = 3
    args = [[np.asarray(inputs[f"{k}{l}"], np.float32) for l in range(L)]
            for k in ("W", "b", "g", "bt", "rm", "rv")]

    key = _inputs_key(inputs)
    if key in _CACHE:
        pl, nc = _CACHE[key]
    else:
        pl = make_plan(x, edge_index, edge_weight, batch, *args)
        nc = build_program(pl)
        _CACHE[key] = (pl, nc)

    res = _run(pl, nc)
    return _assemble(pl, res.results)



# revision 29
# speedup vs baseline: 343.1392x; 343.1392x over previous
"""BrainGNNEncoder (3-layer GCN + BN + ReLU + mean/sum graph pooling) on 8 TRN2 NeuronCores.

Pipeline (per core, SPMD — identical program, per-core data):
  * nodes sharded contiguously; edges sharded by destination node.
  * GCN symmetric norm dinv[src]*|w|*dinv[dst] is a per-edge scalar and is
    folded entirely into the per-slot weight wv (host-side); wv multiplies
    the gathered messages (one [*, H] pass per gather), so the routing
    matrix S is a pure one-hot (single is_equal per supergroup).
  * BatchNorm folded on the host into W'' / b''.
  * per layer:
      dense:  psum2[node, ch] = hT[ch_in, node].T @ W''            (PE)
      writeback (scaled by dinv) bf16 shard -> AllGather -> node
        table [100352, 128] bf16 in DRAM (shared addr space)
      aggregate: dma_gather (int16 idx, 4 SWDGE queues, 4 source
        windows) fetches source rows in LARGE grouped instructions
        (up to MAXBLK_GATHER 128-edge blocks spanning several
        supergroups that share a source window); per 128-edge block a
        matmul msgs[edge, ch].T @ S[edge, 128] accumulates into
        psum1[ch, dst].  S is one-hot built by iota-compare on DVE or
        GpSimd (alternating, to balance engines).
      ScalarE: hT = Relu(psum1*dinv_dst + b'')  (fused BN shift + bias +
        ReLU); last layer also emits per-supergroup column sums via
        accum_out (graph pooling partials).
  * host combines pooling partials (boundary supergroups re-summed from
    the h3 output) into [G, 2H] mean||sum.
"""

import hashlib
import os
import numpy as np
import ml_dtypes

import concourse.bass as bass
import concourse.bacc as bacc
import concourse.tile as tile
import concourse.mybir as mybir
from concourse import library_config
from concourse.bass_utils import run_bass_kernel_spmd

BF16 = ml_dtypes.bfloat16
P = 128            # edge slots per block (matmul contraction dim)
EL = 128           # table row width (bf16) = 256B; cols [H:] are garbage pad
NCORES = 8
EPS = 1e-5
SRCWIN = 32768     # max rows per source window (int16 index range)
MAXBLK_GATHER = int(__import__("os").environ.get("KERNEL_MAXBLK", "8"))  # blocks per dma_gather (1024-idx ucode limit)
WGRP = 4           # supergroups per gather window-group
NQ = 4             # SWDGE queues (hardware max)


class Plan:
    pass


# ----------------------------------------------------------------------------
# Host-side plan
# ----------------------------------------------------------------------------

def make_plan(x, edge_index, edge_weight, batch, Ws, bs, gammas, betas, rms, rvs,
              srcwin=None):
    pl = Plan()
    N, IN = x.shape
    H = Ws[0].shape[1]
    L = len(Ws)
    G = int(batch.max()) + 1 if batch.size else 1
    assert N % NCORES == 0
    NLOC = N // NCORES
    SGN = (NLOC + P - 1) // P
    NLOC_PAD = SGN * P
    TROWS = NLOC_PAD * NCORES
    # writeback/AllGather chunks: sg ranges; each chunk is its own shared
    # table tensor and its own int16 gather window (8 * rows_c < 32768)
    NCHUNK = 4
    bounds = [round(i * SGN / NCHUNK) for i in range(NCHUNK + 1)]
    chunks = [(bounds[i], bounds[i + 1]) for i in range(NCHUNK)]
    assert all(NCORES * (c1 - c0) * P <= 32768 for c0, c1 in chunks)
    NSW = NCHUNK
    pl.chunks = chunks
    pl.N, pl.IN, pl.H, pl.L, pl.G = N, IN, H, L, G
    pl.NLOC, pl.SGN, pl.NLOC_PAD, pl.TROWS, pl.NSW = NLOC, SGN, NLOC_PAD, TROWS, NSW

    # ---- BN folding ----
    Wpp, bpp = [], []
    for l in range(L):
        alpha = (gammas[l] / np.sqrt(rvs[l] + EPS)).astype(np.float32)
        Wpp.append((Ws[l] * alpha[None, :]).astype(np.float32))
        bpp.append(((bs[l] - rms[l]) * alpha + betas[l]).astype(np.float32))
    pl.Wpp = Wpp
    pl.bias_host = np.stack(bpp, axis=1).astype(np.float32)       # [H, L]

    # ---- edges + self loops, factored symmetric norm ----
    src = np.concatenate([edge_index[0], np.arange(N, dtype=np.int64)])
    dst = np.concatenate([edge_index[1], np.arange(N, dtype=np.int64)])
    w = np.concatenate([np.abs(edge_weight), np.ones(N, np.float32)]).astype(np.float32)
    deg = np.zeros(N, np.float32)
    np.add.at(deg, dst, w)
    dinv = np.where(deg > 0,
                    1.0 / np.sqrt(np.where(deg > 0, deg, 1.0)), 0.0).astype(np.float32)

    norm = (dinv[src] * w * dinv[dst]).astype(np.float32)

    core_of = dst // NLOC
    jloc = dst - core_of * NLOC          # local dst id
    sg_of = jloc // P
    # chunk-major table: chunk tensor c holds rows q*rows_c + (r2 - c0*P)
    q2 = src // NLOC
    r2 = src % NLOC
    sg2 = r2 // P
    chunk_start = np.zeros(SGN, np.int64)
    chunk_id = np.zeros(SGN, np.int64)
    for c, (c0, c1) in enumerate(chunks):
        chunk_id[c0:c1] = c
        chunk_start[c0:c1] = c0
    sig_of = chunk_id[sg2]
    rows_c = np.array([(c1 - c0) * P for c0, c1 in chunks], np.int64)
    trow = q2 * rows_c[sig_of] + (r2 - chunk_start[sg2] * P)

    # cell = (sg, sigma); per-cell block counts = max across cores (SPMD)
    NCELL = SGN * NSW
    cellid = sg_of * NSW + sig_of
    counts = np.zeros((NCORES, NCELL), np.int64)
    for q in range(NCORES):
        m = core_of == q
        counts[q] = np.bincount(cellid[m], minlength=NCELL)
    rblk = ((counts.max(axis=0) + P - 1) // P).astype(np.int64)     # [NCELL]
    TOTBLK = int(rblk.sum())
    pl.rblk, pl.TOTBLK = rblk, TOTBLK

    # ---- two block orders over the same physical blocks ----
    # s-order: cells sorted (sg, sigma) — colv layout / S-build / matmuls
    # g-order: cells sorted (window, sigma, sg) — gathers / idx / wv / msgs
    s_base = np.concatenate([[0], np.cumsum(rblk)])[:-1]             # [NCELL]
    NWIN = (SGN + WGRP - 1) // WGRP
    gcell_order = []                    # cell ids in g-order
    for win in range(NWIN):
        for sig in range(NSW):
            for sg in range(win * WGRP, min((win + 1) * WGRP, SGN)):
                gcell_order.append(sg * NSW + sig)
    gcell_order = np.array(gcell_order, np.int64)
    g_base_by_pos = np.concatenate([[0], np.cumsum(rblk[gcell_order])])[:-1]
    g_base = np.zeros(NCELL, np.int64)
    g_base[gcell_order] = g_base_by_pos
    # map s-order block id -> g-order block id
    g_of_s = np.zeros(TOTBLK, np.int64)
    for ci in range(NCELL):
        nb = int(rblk[ci])
        g_of_s[s_base[ci]:s_base[ci] + nb] = np.arange(
            g_base[ci], g_base[ci] + nb)
    pl.g_of_s = g_of_s

    # gather instructions over g-order: per (window, sigma) run, chunks of
    # <= MAXBLK_GATHER blocks.  (sig, gb0, nblk, idxcol0, win)
    instrs = []
    idxcol = 0
    for win in range(NWIN):
        for sig in range(NSW):
            cells = [sg * NSW + sig
                     for sg in range(win * WGRP, min((win + 1) * WGRP, SGN))]
            nb = int(sum(rblk[c] for c in cells))
            if nb == 0:
                continue
            gb0 = int(g_base[cells[0]])
            off = 0
            while off < nb:
                step = min(MAXBLK_GATHER, nb - off)
                instrs.append((sig, gb0 + off, step, idxcol, win))
                idxcol += step * 8
                off += step
    pl.instrs = instrs
    pl.IDXCOLS = idxcol
    pl.NWIN = NWIN

    # s-order block range of each supergroup (cells (sg, sig) contiguous)
    pl.sg_blk = [(int(s_base[sg * NSW]),
                  int(s_base[sg * NSW] + sum(rblk[sg * NSW + s]
                                             for s in range(NSW))))
                 for sg in range(SGN)]

    # ---- per-core packed data ----
    pl.idx_data, pl.colv_data, pl.wv_data = [], [], []
    pl.xT_data = []
    order = np.lexsort((jloc, cellid, core_of))   # sort by core, cell, dst
    src_s = trow[order]
    col_s = (jloc % P)[order]
    w_s = norm[order]
    cell_s = cellid[order]
    core_s = core_of[order]
    for q in range(NCORES):
        m = core_s == q
        cq, rq, colq, wq = cell_s[m], src_s[m], col_s[m], w_s[m]
        off = np.concatenate([[0], np.cumsum(counts[q])])[:-1]
        pos = np.arange(len(cq)) - off[cq]
        sblk_e = s_base[cq] + pos // P
        gblk_e = g_base[cq] + pos // P
        p_e = pos % P

        # pads fetch row 0 of the chunk table (cheap, finite data); padded
        # slots have wv == 0 so they contribute nothing.
        idx16 = np.zeros((TOTBLK, P), np.int16)
        idx16[gblk_e, p_e] = rq.astype(np.int16)
        colv = np.zeros((TOTBLK, P), np.float32)
        colv[sblk_e, p_e] = colq
        wv = np.zeros((TOTBLK, P), np.float32)
        wv[gblk_e, p_e] = wq

        # wrapped idx stream per gather instruction (g-order)
        idxw = np.zeros((P, idxcol), np.int16)
        for (sig, gb0, nblk, c0, win) in instrs:
            flat = idx16[gb0:gb0 + nblk].reshape(-1)
            idxw[:, c0:c0 + nblk * 8] = np.tile(
                flat.reshape(nblk * 8, 16).T, (NCORES, 1))
        pl.idx_data.append(idxw)
        pl.colv_data.append(np.ascontiguousarray(colv.T).astype(BF16))  # [P, TOTBLK]
        pl.wv_data.append(np.ascontiguousarray(wv.T).astype(BF16))      # [P, TOTBLK]

        xs = x[q * NLOC:(q + 1) * NLOC]
        xT = np.zeros((IN, NLOC_PAD), np.float32)
        xT[:, :NLOC] = xs.T
        pl.xT_data.append(xT.astype(BF16))

    # ---- pooling bookkeeping ----
    gcnt = np.bincount(batch, minlength=G).astype(np.int64)
    gstart = np.concatenate([[0], np.cumsum(gcnt)])
    pl.gcnt = gcnt
    pieces = []
    for q in range(NCORES):
        lo = q * NLOC
        per_sg = []
        for sg in range(SGN):
            s0, s1 = sg * P, min(sg * P + P, NLOC)
            segs = []
            c = s0
            g0 = int(np.searchsorted(gstart, lo + c, side="right") - 1)
            while c < s1:
                g_end = int(gstart[g0 + 1]) - lo
                e = min(s1, g_end)
                segs.append((c - s0, e - s0, g0))
                c = e
                if c >= g_end:
                    g0 += 1
            per_sg.append(segs)
        pieces.append(per_sg)
    pl.pieces = pieces
    return pl


# ----------------------------------------------------------------------------
# Program builder
# ----------------------------------------------------------------------------

def build_program(pl, repeats=1):
    dt = mybir.dt
    f32, bf16, i16 = dt.float32, dt.bfloat16, dt.int16
    IN, H, SGN, NLOC_PAD, TOTBLK, L = pl.IN, pl.H, pl.SGN, pl.NLOC_PAD, pl.TOTBLK, pl.L
    TROWS = pl.TROWS

    nc = bacc.Bacc("TRN2", target_bir_lowering=False, debug=False,
                   num_devices=NCORES, num_swdge_queues=NQ,
                   dynamic_dma_scratch_size=16 * MAXBLK_GATHER * P)

    xT_d = nc.dram_tensor("xT", [IN, NLOC_PAD], bf16, kind="ExternalInput")
    idx_d = nc.dram_tensor("idx", [P, pl.IDXCOLS], i16, kind="ExternalInput")
    colv_d = nc.dram_tensor("colv", [P, TOTBLK], bf16, kind="ExternalInput")
    wv_d = nc.dram_tensor("wv", [P, TOTBLK], bf16, kind="ExternalInput")
    iota_d = nc.dram_tensor("iota", [P, P], bf16, kind="ExternalInput")
    W_d = [nc.dram_tensor(f"W{l}", [IN if l == 0 else H, H], bf16,
                          kind="ExternalInput") for l in range(L)]
    bias_d = nc.dram_tensor("bias", [H, L], f32, kind="ExternalInput")
    pool_d = nc.dram_tensor("pool", [H, SGN], f32, kind="ExternalOutput")
    h3_d = nc.dram_tensor("h3", [H, NLOC_PAD], bf16, kind="ExternalOutput")

    rg = [list(range(NCORES))]

    with tile.TileContext(nc) as tc:
        with (
            tc.tile_pool(name="const", bufs=1) as constp,
            tc.tile_pool(name="xtp", bufs=1) as xtp,
            tc.tile_pool(name="dram", bufs=1, space="DRAM") as dramp,
            tc.tile_pool(name="msgs", bufs=int(__import__("os").environ.get("KERNEL_MSGBUFS", "14"))) as msgp,
            tc.tile_pool(name="sbld", bufs=3) as sbldp,
            tc.tile_pool(name="aggp", bufs=4) as aggp,
            tc.tile_pool(name="ps1p", bufs=4, space="PSUM") as ps1p,
            tc.tile_pool(name="ps2p", bufs=4, space="PSUM") as ps2p,
        ):
            nc.gpsimd.load_library(library_config.mlp)

            MAXL = int(os.environ.get("KERNEL_MAXL", str(L)))
            NO_GATHER = os.environ.get("KERNEL_NO_GATHER") == "1"
            NO_SBUILD = os.environ.get("KERNEL_NO_SBUILD") == "1"
            NO_AGGMM = os.environ.get("KERNEL_NO_AGGMM") == "1"
            # fraction of supergroups whose S-build runs on GpSimd
            # (neuronxcc only lowers Add/Multiply on Pool, so default 0)
            GP_FRAC = float(os.environ.get("KERNEL_GP_FRAC", "0"))
            SINGLE_PACKET = os.environ.get("KERNEL_SINGLE_PACKET", "1") == "1"
            FOLD_GP_FRAC = float(os.environ.get("KERNEL_FOLD_GP_FRAC", "0"))

            for rep in range(repeats):
                body_once(pl, nc, tc, constp, xtp, dramp, msgp, sbldp, aggp,
                          ps1p, ps2p, xT_d, idx_d, colv_d, wv_d, iota_d,
                          W_d, bias_d, pool_d, h3_d, rg,
                          rep, MAXL, NO_GATHER, NO_SBUILD, NO_AGGMM,
                          GP_FRAC, SINGLE_PACKET, FOLD_GP_FRAC)

    nc.compile()
    return nc


def body_once(pl, nc, tc, constp, xtp, dramp, msgp, sbldp, aggp, ps1p, ps2p,
              xT_d, idx_d, colv_d, wv_d, iota_d, W_d,
              bias_d, pool_d, h3_d, rg, rep, MAXL, NO_GATHER, NO_SBUILD,
              NO_AGGMM, GP_FRAC, SINGLE_PACKET, FOLD_GP_FRAC):
    dt = mybir.dt
    f32, bf16, i16 = dt.float32, dt.bfloat16, dt.int16
    IN, H, SGN, NLOC_PAD, TOTBLK, L = pl.IN, pl.H, pl.SGN, pl.NLOC_PAD, pl.TOTBLK, pl.L
    TROWS = pl.TROWS
    sfx = f"_r{rep}"

    # ---------------- constants ----------------
    idx_sb = constp.tile([P, pl.IDXCOLS], i16, name="idx_sb" + sfx, tag="idx_sb")
    nc.sync.dma_start(out=idx_sb[:], in_=idx_d[:, :])
    colv_sb = constp.tile([P, TOTBLK], bf16, name="colv_sb" + sfx, tag="colv_sb")
    nc.sync.dma_start(out=colv_sb[:], in_=colv_d[:, :])
    wv_sb = constp.tile([P, TOTBLK], bf16, name="wv_sb" + sfx, tag="wv_sb")
    nc.sync.dma_start(out=wv_sb[:], in_=wv_d[:, :])
    iota_sb = constp.tile([P, P], bf16, name="iota_sb" + sfx, tag="iota_sb")
    nc.sync.dma_start(out=iota_sb[:], in_=iota_d[:, :])
    W_sb = []
    for l in range(L):
        wt = constp.tile([IN if l == 0 else H, H], bf16,
                         name=f"W{l}_sb" + sfx, tag=f"W{l}_sb")
        nc.sync.dma_start(out=wt[:], in_=W_d[l][:])
        W_sb.append(wt)
    bias_sb = constp.tile([H, L], f32, name="bias_sb" + sfx, tag="bias_sb")
    nc.sync.dma_start(out=bias_sb[:], in_=bias_d[:])
    xT_sb = xtp.tile([IN, NLOC_PAD], bf16, name="xT_sb" + sfx, tag="xT_sb")
    nc.sync.dma_start(out=xT_sb[:], in_=xT_d[:, :])

    chunks = pl.chunks
    wbc = [constp.tile([P, (c1 - c0) * H], bf16, name=f"wb{c}" + sfx,
                       tag=f"wb{c}")
           for c, (c0, c1) in enumerate(chunks)]
    h3T = constp.tile([H, NLOC_PAD], bf16, name="h3T" + sfx, tag="h3T")
    pool_sb = constp.tile([H, SGN], f32, name="pool_sb" + sfx, tag="pool_sb")
    nc.vector.memset(h3T[:], 0.0)
    nc.vector.memset(pool_sb[:], 0.0)

    bounce = [[dramp.tile([(c1 - c0) * P, EL], bf16,
                          name=f"bounce{l}_{c}" + sfx, tag=f"bounce{l}_{c}")
               for c, (c0, c1) in enumerate(chunks)] for l in range(L)]
    tables = [[dramp.tile([NCORES * (c1 - c0) * P, EL], bf16,
                          addr_space="Shared",
                          name=f"T{l}_{c}" + sfx, tag=f"T{l}_{c}")
               for c, (c0, c1) in enumerate(chunks)] for l in range(L)]

    def writeback_and_allgather(l, c):
        # chunk c of the (chunk-major) node table: bounce rows (sg-c0)*P + p
        bc = bounce[l][c]
        dview = bc.rearrange("(s p) h -> p s h", p=P)[:, :, :H]
        nc.sync.dma_start(
            out=dview, in_=wbc[c][:].rearrange("p (s h) -> p s h", h=H))
        nc.gpsimd.collective_compute(
            "AllGather", mybir.AluOpType.bypass,
            replica_groups=rg,
            ins=[bc[:, :].opt()],
            outs=[tables[l][c][:, :].opt()],
        )

    def chunk_of(sg):
        for c, (c0, c1) in enumerate(chunks):
            if c0 <= sg < c1:
                return c, c0, c1
        raise AssertionError(sg)

    # ---------------- phase D0: dense layer 0 (dinv-scaled writeback) ----
    for t in range(SGN):
        c, c0, c1 = chunk_of(t)
        ps2 = ps2p.tile([P, H], f32, tag="ps2", name=f"ps2_d0_{t}" + sfx)
        nc.tensor.matmul(out=ps2[:], lhsT=xT_sb[:, t * P:(t + 1) * P],
                         rhs=W_sb[0][:], start=True, stop=True)
        nc.scalar.activation(out=wbc[c][:, (t - c0) * H:(t - c0 + 1) * H],
                             in_=ps2[:],
                             func=mybir.ActivationFunctionType.Copy)
        if t == c1 - 1:
            writeback_and_allgather(0, c)

    # ---------------- layers ----------------
    for l in range(min(L, MAXL)):
        gi = 0           # next gather instruction to issue
        msgs_of = {}     # gblk -> (tile, chunk)
        for sg in range(SGN):
            win = sg // WGRP
            # issue gathers for this window and the next (prefetch)
            while gi < len(pl.instrs) and pl.instrs[gi][4] <= win + 1:
                sig, gb0, nblk, c0, _w = pl.instrs[gi]
                m = msgp.tile([P, MAXBLK_GATHER, EL], bf16, tag="msgs",
                              name=f"msgs_{l}_{gi}" + sfx)
                if NO_GATHER:
                    nc.vector.memset(m[:1, :1, :1], 0.0)
                else:
                    nc.gpsimd.dma_gather(
                        out_ap=m[:, :nblk, :],
                        in_ap=tables[l][sig][0:, :],
                        idxs_ap=idx_sb[:, c0:c0 + nblk * 8],
                        num_idxs=nblk * P, num_idxs_reg=nblk * P,
                        elem_size=EL, queue_num=gi % NQ,
                        single_packet=SINGLE_PACKET)
                    # fold |w| into the gathered messages (per-slot scale)
                    wv_b = wv_sb[:, gb0:gb0 + nblk, None].to_broadcast(
                        [P, nblk, H])
                    feng = nc.gpsimd if (gi % 10) < FOLD_GP_FRAC * 10 \
                        else nc.vector
                    feng.tensor_tensor(
                        out=m[:, :nblk, :H], in0=m[:, :nblk, :H], in1=wv_b,
                        op=mybir.AluOpType.mult)
                for k in range(nblk):
                    msgs_of[gb0 + k] = (m, k)
                gi += 1
            s_lo, s_hi = pl.sg_blk[sg]
            nbs = s_hi - s_lo
            if nbs == 0:
                continue
            # one-hot S for this supergroup's blocks (s-order, contiguous)
            S_sb = sbldp.tile([P, nbs, P], bf16, tag="S_sb",
                              name=f"S_{l}_{sg}" + sfx)
            iota_b = bass.AP(iota_sb[:].tensor, iota_sb[:].offset,
                             [iota_sb[:].ap[0], [0, nbs],
                              iota_sb[:].ap[1]])
            colv_b = colv_sb[:, s_lo:s_hi, None].to_broadcast([P, nbs, P])
            if NO_SBUILD:
                nc.vector.memset(S_sb[:1, :1, :1], 0.0)
            else:
                eng = nc.gpsimd if (sg % 10) < GP_FRAC * 10 else nc.vector
                eng.tensor_tensor(out=S_sb[:], in0=iota_b, in1=colv_b,
                                  op=mybir.AluOpType.is_equal)
            ps1 = ps1p.tile([H, P], f32, tag="ps1",
                            name=f"ps1_{l}_{sg}" + sfx)
            if NO_AGGMM:
                nc.vector.memset(ps1[:1, :1], 0.0)
                for sb in range(s_lo, s_hi):
                    msgs_of.pop(int(pl.g_of_s[sb]))
            else:
                for j, sb in enumerate(range(s_lo, s_hi)):
                    m, k = msgs_of.pop(int(pl.g_of_s[sb]))
                    nc.tensor.matmul(
                        out=ps1[:, :],
                        lhsT=m[:, k, :H],
                        rhs=S_sb[:, j, :],
                        start=(j == 0), stop=(j == nbs - 1))
            if l < L - 1:
                aggT = aggp.tile([H, P], bf16, tag="aggT",
                                 name=f"aggT_{l}_{sg}" + sfx)
                nc.scalar.activation(
                    out=aggT[:], in_=ps1[:],
                    func=mybir.ActivationFunctionType.Relu,
                    bias=bias_sb[:, l:l + 1], scale=1.0)
                ps2 = ps2p.tile([P, H], f32, tag="ps2",
                                name=f"ps2_{l}_{sg}" + sfx)
                nc.tensor.matmul(out=ps2[:], lhsT=aggT[:],
                                 rhs=W_sb[l + 1][:],
                                 start=True, stop=True)
                c, c0, c1 = chunk_of(sg)
                nc.scalar.activation(
                    out=wbc[c][:, (sg - c0) * H:(sg - c0 + 1) * H],
                    in_=ps2[:],
                    func=mybir.ActivationFunctionType.Copy)
                if sg == c1 - 1:
                    writeback_and_allgather(l + 1, c)
            else:
                nc.scalar.activation(
                    out=h3T[:, sg * P:(sg + 1) * P], in_=ps1[:],
                    func=mybir.ActivationFunctionType.Relu,
                    bias=bias_sb[:, l:l + 1], scale=1.0,
                    accum_out=pool_sb[:, sg:sg + 1])

    nc.sync.dma_start(out=pool_d[:, :], in_=pool_sb[:])
    nc.sync.dma_start(out=h3_d[:, :], in_=h3T[:])


# ----------------------------------------------------------------------------
# kernel entry point
# ----------------------------------------------------------------------------

_CACHE = {}


def _inputs_key(inputs):
    h = hashlib.sha1()
    for k in sorted(inputs.keys()):
        a = np.asarray(inputs[k])
        h.update(k.encode())
        h.update(str(a.shape).encode())
    h.update(np.ascontiguousarray(np.asarray(inputs["edge_index"], np.int64)).tobytes())
    h.update(np.ascontiguousarray(np.asarray(inputs["batch"], np.int64)).tobytes())
    return h.hexdigest()


def _in_maps(pl):
    iota = np.broadcast_to(np.arange(P, dtype=np.float32), (P, P))
    iota = np.ascontiguousarray(iota).astype(BF16)
    in_maps = []
    for q in range(NCORES):
        in_map = {
            "xT": pl.xT_data[q],
            "idx": pl.idx_data[q],
            "colv": pl.colv_data[q],
            "wv": pl.wv_data[q],
            "iota": iota,
            "bias": pl.bias_host,
        }
        for l in range(pl.L):
            in_map[f"W{l}"] = pl.Wpp[l].astype(BF16)
        in_maps.append(in_map)
    return in_maps


def _run(pl, nc):
    return run_bass_kernel_spmd(nc, _in_maps(pl), core_ids=list(range(NCORES)))


def _assemble(pl, results):
    G, H, SGN, P_ = pl.G, pl.H, pl.SGN, P
    sums = np.zeros((G, H), np.float64)
    for q in range(NCORES):
        pool = np.asarray(results[q]["pool"], np.float64)   # [H, SGN]
        h3 = np.asarray(results[q]["h3"], np.float32)       # [H, NLOC_PAD]
        for sg in range(SGN):
            segs = pl.pieces[q][sg]
            if len(segs) == 1 and segs[0][0] == 0 and segs[0][1] == P_:
                sums[segs[0][2]] += pool[:, sg]
            else:
                for (c0, c1, g) in segs:
                    sums[g] += h3[:, sg * P_ + c0: sg * P_ + c1].astype(
                        np.float64).sum(axis=1)
    cnt = np.maximum(pl.gcnt, 1).astype(np.float64)
    mean = sums / cnt[:, None]
    return np.concatenate([mean, sums], axis=1).astype(np.float32)


def kernel(**inputs) -> np.ndarray:
    x = np.asarray(inputs["x"], np.float32)
    edge_index = np.asarray(inputs["edge_index"]).astype(np.int64)
    edge_weight = np.asarray(inputs["edge_weight"], np.float32)
    batch = np.asarray(inputs["batch"]).astype(np.int64)
    L = 3
    args = [[np.asarray(inputs[f"{k}{l}"], np.float32) for l in range(L)]
            for k in ("W", "b", "g", "bt", "rm", "rv")]

    key = _inputs_key(inputs)
    if key in _CACHE:
        pl, nc = _CACHE[key]
    else:
        pl = make_plan(x, edge_index, edge_weight, batch, *args)
        nc = build_program(pl)
        _CACHE[key] = (pl, nc)

    res = _run(pl, nc)
    return _assemble(pl, res.results)
